# revision 11
# baseline (speedup 1.0000x reference)
"""Causal self-attention kernel for Trainium2, sharded over 8 NeuronCores.

Problem (hardcoded): x [2, 2048, 1024] fp32, Wq/Wk/Wv/Wo [1024, 1024], bo [1024].
H = 16 heads, head dim 64.

Sharding: tensor-parallel over heads. Each core owns 2 heads (a 128-wide
column slice of Wq/Wk/Wv and the matching 128-row slice of Wo), computes its
partial out-projection y_i = ctx_i @ Wo[rows_i], and the host sums the 8
partials (the "all-reduce") and adds bo.

Everything on-device is kept in a transposed ("feature-major") layout so that
no on-device transposes of activations or attention probabilities are needed:

  qT, kT        [128 (2 heads x 64 d), T]   from  W.T-slices as lhsT, xT as rhs
  scoresT       [k-tile 128, 2 heads, q-chunk 512] in one 2-bank PSUM tile;
                the two heads' K=64 matmuls sit at base partitions 0/64 so the
                PE row-groups run them concurrently
  expT          exp(scoresT / 8) for both heads in ONE ScalarE activation;
                causal-masked on GPSIMD via a triangular 0/1 mask
  ctxT (+l)     [65, 512]: lhsT = [v | ones] so row 64 accumulates the softmax
                denominator for free; ctxT /= l via partition-broadcast recip
  y partial     lhsT = normalized ctxT t-tile, rhs = Wo local rows

Matmuls use float32r (full-rate fp32 at free dim >= 256).
"""

import sys

import numpy as np

try:
    import concourse.bass as bass  # noqa: F401
except ImportError:  # harness environments without concourse on sys.path
    sys.path.insert(0, "/opt/trn_rl_repo")
    import concourse.bass as bass  # noqa: F401

from contextlib import ExitStack

import concourse.mybir as mybir
import concourse.tile as tile
from concourse import bacc
from concourse.bass import ts
from concourse.bass_utils import run_bass_kernel_spmd

F32 = mybir.dt.float32
F32R = mybir.dt.float32r
F16 = mybir.dt.float16

N_CORES = 8
B, S, E = 2, 2048, 1024
H, D = 16, 64
EL = 128          # local e' width per core (2 heads x 64)
CH = 512          # q-chunk width (one PSUM bank of fp32)
KT = 128          # k-tile width
NE = E // 128     # e-tiles in the contraction dim


def build_attention(batch=B, seq=S, dt_in=F32, dt_out=F32):
    """Build the per-core Bass program (same program on all 8 cores)."""
    T = batch * seq
    ncb = seq // CH            # q-chunks per batch
    ntt = seq // 128           # t-tiles per batch
    nkt_b = seq // KT          # k-tiles per batch

    nc = bacc.Bacc("TRN2", debug=False, num_devices=N_CORES)

    dt_ind = F32R if dt_in == F32 else dt_in
    xT = nc.dram_tensor("xT", [E, T], dt_ind, kind="ExternalInput").ap()
    wq = nc.dram_tensor("wq", [128, E], dt_ind, kind="ExternalInput").ap()
    wk = nc.dram_tensor("wk", [128, E], dt_ind, kind="ExternalInput").ap()
    wv = nc.dram_tensor("wv", [128, E], dt_ind, kind="ExternalInput").ap()
    wo = nc.dram_tensor("wo", [EL, E], F32R, kind="ExternalInput").ap()
    trimask = nc.dram_tensor("trimask", [128, 128], F32, kind="ExternalInput").ap()
    ident = nc.dram_tensor("ident", [128, 128], F32R, kind="ExternalInput").ap()
    onesc = nc.dram_tensor("onesc", [128, 1], F32R, kind="ExternalInput").ap()
    y = nc.dram_tensor("y", [batch, seq, E], dt_out, kind="ExternalOutput").ap()

    with tile.TileContext(nc) as tc, ExitStack() as ctx:
        consts = ctx.enter_context(tc.tile_pool(name="consts", bufs=1))
        xt_pool = ctx.enter_context(tc.tile_pool(name="xt", bufs=NE + 2))
        big = ctx.enter_context(tc.tile_pool(name="big", bufs=2))
        vsb_pool = ctx.enter_context(tc.tile_pool(name="vsb", bufs=2))
        ex_pool = ctx.enter_context(tc.tile_pool(name="ex", bufs=4))
        ysb_pool = ctx.enter_context(tc.tile_pool(name="ysb", bufs=3))
        small = ctx.enter_context(tc.tile_pool(name="small", bufs=2))
        # PSUM (8 banks): scores 2x2 + ctx 2x1 + shared proj/transpose/y 2x1
        sc_pool = ctx.enter_context(tc.tile_pool(name="scps", bufs=2, space="PSUM"))
        ctx_pool = ctx.enter_context(tc.tile_pool(name="ctxps", bufs=2, space="PSUM"))
        mm_pool = ctx.enter_context(tc.tile_pool(name="mmps", bufs=2, space="PSUM"))

        # ---- constants / weights (resident) ----
        wq_sb = consts.tile([128, E], dt_ind)
        wk_sb = consts.tile([128, E], dt_ind)
        wv_sb = consts.tile([128, E], dt_ind)
        wo_sb = consts.tile([EL, E], F32R)
        tri_sb = consts.tile([128, 128], F32)
        id_sb = consts.tile([128, 128], F32R)
        ones_sb = consts.tile([128, 1], F32R)
        for dst, src in ((wq_sb, wq), (wk_sb, wk), (wv_sb, wv), (wo_sb, wo),
                         (tri_sb, trimask), (id_sb, ident), (ones_sb, onesc)):
            nc.sync.dma_start(dst[:], src)

        n_y = 0
        for b in range(batch):
            # ---- load xT for this batch ----
            xts = []
            for e in range(NE):
                xt = xt_pool.tile([128, seq], dt_ind, tag="xt")
                nc.sync.dma_start(xt[:], xT[e * 128:(e + 1) * 128, b * seq:(b + 1) * seq])
                xts.append(xt)

            # ---- per-chunk: project q/k/v chunk qc, then attend qc ----
            qT = big.tile([EL, seq], F32R, tag="qT")
            kT = big.tile([EL, seq], F32R, tag="kT")
            vT = big.tile([EL, seq], F32R, tag="vT", bufs=1)
            ctxnT = big.tile([EL, seq], F32R, tag="ctxnT", bufs=1)
            vsb_all = vsb_pool.tile([128, nkt_b, 130], F32R, tag="vsb", name="vsb_all")
            nc.vector.tensor_copy(
                vsb_all.rearrange("p t (h c) -> p (t h) c", c=65)[:, :, 64:65],
                ones_sb.broadcast_to([128, 2 * nkt_b, 1]))
            for qc in range(ncb):
                # q and k projections for this chunk share one 2-bank sc tile
                psqk = sc_pool.tile([128, 2, CH], F32, tag="sc", name="psqk")
                for e in range(NE):
                    nc.tensor.matmul(psqk[:, 0, :], wq_sb[:, ts(e, 128)],
                                     xts[e][:, ts(qc, CH)],
                                     start=(e == 0), stop=(e == NE - 1))
                    nc.tensor.matmul(psqk[:, 1, :], wk_sb[:, ts(e, 128)],
                                     xts[e][:, ts(qc, CH)],
                                     start=(e == 0), stop=(e == NE - 1))
                nc.vector.tensor_copy(qT[:, ts(qc, CH)], psqk[:, 0, :])
                nc.vector.tensor_copy(kT[:, ts(qc, CH)], psqk[:, 1, :])
                # v projection for this chunk
                psv = mm_pool.tile([128, CH], F32, tag="mm", name="psv")
                for e in range(NE):
                    nc.tensor.matmul(psv[:], wv_sb[:, ts(e, 128)],
                                     xts[e][:, ts(qc, CH)],
                                     start=(e == 0), stop=(e == NE - 1))
                nc.vector.tensor_copy(vT[:, ts(qc, CH)], psv[:])
                # transpose this chunk's 4 v t-tiles into [k, (vA|1|vB|1)]
                for tt in range(qc * (CH // 128), (qc + 1) * (CH // 128)):
                    tp = mm_pool.tile([128, 128], F32R, tag="mm", name="tp_ps")
                    nc.tensor.transpose(tp[:], vT[:, ts(tt, 128)], id_sb[:])
                    nc.vector.tensor_copy(
                        vsb_all[:, tt, 0:130].rearrange("p (h c) -> p h c", h=2)[:, :, 0:64],
                        tp.rearrange("p (h c) -> p h c", h=2),
                    )

                nkt = (qc * CH + CH) // KT  # causal: k-tiles 0 .. nkt-1
                cps = [ctx_pool.tile([65, CH], F32, tag="ctx", name=f"ctx_ps{hi}")
                       for hi in range(2)]
                for kt in range(nkt):
                    c0 = max(0, kt * KT - qc * CH)
                    n = CH - c0
                    diag = kt * KT >= qc * CH
                    # both heads' scores into one 2-bank psum tile
                    sc = sc_pool.tile([128, 2, CH], F32, tag="sc", name="sc_ps")
                    for hi in range(2):
                        r0 = hi * 64
                        nc.tensor.matmul(
                            sc[:, hi, 0:n],
                            kT[r0:r0 + 64, ts(kt, KT)],
                            qT[r0:r0 + 64, qc * CH + c0:(qc + 1) * CH],
                            start=True, stop=True,
                        )
                    ex = ex_pool.tile([128, 2, CH], F32R, tag="ex", name="ex")
                    nc.scalar.activation(
                        ex[:, :, 0:n], sc[:, :, 0:n],
                        mybir.ActivationFunctionType.Exp, scale=1.0 / np.sqrt(D))
                    if diag:
                        nc.gpsimd.tensor_mul(
                            ex[:, :, 0:128], ex[:, :, 0:128],
                            tri_sb.unsqueeze(1).broadcast_to([128, 2, 128]))
                    for hi in range(2):
                        nc.tensor.matmul(
                            cps[hi][:, c0:CH],
                            vsb_all[:, kt, hi * 65:(hi + 1) * 65],
                            ex[:, hi, 0:n],
                            start=(kt == 0), stop=(kt == nkt - 1),
                            skip_group_check=True,
                        )
                # normalize: ctxT[0:64] / l (row 64)
                for hi in range(2):
                    r0 = hi * 64
                    rc = small.tile([1, CH], F32, tag="rc", name="rc")
                    nc.vector.reciprocal(rc[:], cps[hi][64:65, :])
                    bc = small.tile([64, CH], F32, tag="bc", name="bc")
                    nc.gpsimd.partition_broadcast(bc[:], rc[:])
                    nc.vector.tensor_mul(
                        ctxnT[r0:r0 + 64, ts(qc, CH)], cps[hi][0:64, :], bc[:])

                # out-projection for this chunk's 4 t-tiles
                for tt in range(qc * (CH // 128), (qc + 1) * (CH // 128)):
                    for eo in range(E // CH):
                        yp = mm_pool.tile([128, CH], F32, tag="mm", name="y_ps")
                        nc.tensor.matmul(
                            yp[:], ctxnT[:, ts(tt, 128)], wo_sb[:, ts(eo, CH)],
                            start=True, stop=True)
                        ysb = ysb_pool.tile([128, CH], dt_out, tag="ysb", name="ysb")
                        if n_y % 2 == 0:
                            nc.vector.tensor_copy(ysb[:], yp[:])
                        else:
                            nc.scalar.copy(ysb[:], yp[:])
                        n_y += 1
                        nc.sync.dma_start(
                            y[b, tt * 128:(tt + 1) * 128, eo * CH:(eo + 1) * CH],
                            ysb[:])

    nc.compile()
    return nc


def _prep_inputs(x, Wq, Wk, Wv, Wo, dt_in=np.float32):
    """Host-side sharding: transpose x, slice weights per core."""
    batch, seq, _ = x.shape
    xT = np.ascontiguousarray(x.reshape(batch * seq, E).T).astype(dt_in)
    tri = np.triu(np.ones((128, 128), np.float32))  # tri[p, c] = 1 iff p <= c
    identity = np.eye(128, dtype=np.float32)

    def warr(w):  # [E, 128] col-slice -> SBUF layout [128, 8*128]
        return np.ascontiguousarray(
            w.reshape(NE, 128, 128).transpose(1, 0, 2).reshape(128, E)
        ).astype(dt_in)

    in_maps = []
    for i in range(N_CORES):
        cols = slice(i * EL, (i + 1) * EL)
        in_maps.append({
            "xT": xT,
            "wq": warr(Wq[:, cols]),
            "wk": warr(Wk[:, cols]),
            "wv": warr(Wv[:, cols]),
            "wo": np.ascontiguousarray(Wo[cols, :]).astype(np.float32),
            "trimask": tri,
            "ident": identity,
            "onesc": np.ones((128, 1), np.float32),
        })
    return in_maps


_CACHE = {}


def _get_nc(batch, seq, dt_in, dt_out):
    key = (batch, seq, dt_in, dt_out)
    if key not in _CACHE:
        _CACHE[key] = build_attention(batch, seq, dt_in, dt_out)
    return _CACHE[key]


DT_IN = F16   # fp16 x/W transfers; projections accumulate fp32 in PSUM
DT_OUT = F16  # fp16 partial-y transfers; host sums in fp32


def kernel(x, Wq, Wk, Wv, Wo, bo, _trace=False):
    x = np.asarray(x, np.float32)
    batch, seq, _ = x.shape
    nc = _get_nc(batch, seq, DT_IN, DT_OUT)
    in_maps = _prep_inputs(x, np.asarray(Wq), np.asarray(Wk), np.asarray(Wv),
                           np.asarray(Wo),
                           dt_in=np.float16 if DT_IN == F16 else np.float32)
    res = run_bass_kernel_spmd(nc, in_maps, core_ids=list(range(N_CORES)),
                               trace=_trace)
    parts = [res.results[i]["y"].astype(np.float32) for i in range(N_CORES)]
    y = np.sum(parts, axis=0, dtype=np.float32) + np.asarray(bo, np.float32)
    if _trace:
        kernel.last_results = res
    return y


# revision 12
# speedup vs baseline: 12823.9076x; 12823.9076x over previous
"""Causal self-attention kernel for Trainium2, sharded over 8 NeuronCores.

Problem (hardcoded): x [2, 2048, 1024] fp32, Wq/Wk/Wv/Wo [1024, 1024], bo [1024].
H = 16 heads, head dim 64.

Sharding: tensor-parallel over heads. Each core owns 2 heads (a 128-wide
column slice of Wq/Wk/Wv and the matching 128-row slice of Wo), computes its
partial out-projection y_i = ctx_i @ Wo[rows_i], and the host sums the 8
partials (the "all-reduce") and adds bo.

Everything on-device is kept in a transposed ("feature-major") layout so that
no on-device transposes of activations or attention probabilities are needed:

  qT, kT        [128 (2 heads x 64 d), T]   from  W.T-slices as lhsT, xT as rhs
  scoresT       [k-tile 128, 2 heads, q-chunk 512] in one 2-bank PSUM tile;
                the two heads' K=64 matmuls sit at base partitions 0/64 so the
                PE row-groups run them concurrently
  expT          exp(scoresT / 8) for both heads in ONE ScalarE activation;
                causal-masked on GPSIMD via a triangular 0/1 mask
  ctxT (+l)     [65, 512]: lhsT = [v | ones] so row 64 accumulates the softmax
                denominator for free; ctxT /= l via partition-broadcast recip
  y partial     lhsT = normalized ctxT t-tile, rhs = Wo local rows

Matmuls use float32r (full-rate fp32 at free dim >= 256).
"""

import sys

import numpy as np

try:
    import concourse.bass as bass  # noqa: F401
except ImportError:  # harness environments without concourse on sys.path
    sys.path.insert(0, "/opt/trn_rl_repo")
    import concourse.bass as bass  # noqa: F401

from contextlib import ExitStack

import concourse.mybir as mybir
import concourse.tile as tile
from concourse import bacc
from concourse.bass import ts
from concourse.bass_utils import run_bass_kernel_spmd

F32 = mybir.dt.float32
F32R = mybir.dt.float32r
F16 = mybir.dt.float16

N_CORES = 8
B, S, E = 2, 2048, 1024
H, D = 16, 64
EL = 128          # local e' width per core (2 heads x 64)
CH = 512          # q-chunk width (one PSUM bank of fp32)
KT = 128          # k-tile width
NE = E // 128     # e-tiles in the contraction dim


def build_attention(batch=B, seq=S, dt_in=F32, dt_out=F32, n_reps=1):
    """Build the per-core Bass program (same program on all 8 cores)."""
    T = batch * seq
    ncb = seq // CH            # q-chunks per batch
    ntt = seq // 128           # t-tiles per batch
    nkt_b = seq // KT          # k-tiles per batch

    nc = bacc.Bacc("TRN2", debug=False, num_devices=N_CORES)

    dt_ind = F32R if dt_in == F32 else dt_in
    xT = nc.dram_tensor("xT", [E, T], dt_ind, kind="ExternalInput").ap()
    wq = nc.dram_tensor("wq", [128, E], dt_ind, kind="ExternalInput").ap()
    wk = nc.dram_tensor("wk", [128, E], dt_ind, kind="ExternalInput").ap()
    wv = nc.dram_tensor("wv", [128, E], dt_ind, kind="ExternalInput").ap()
    wo = nc.dram_tensor("wo", [EL, E], F32R, kind="ExternalInput").ap()
    trimask = nc.dram_tensor("trimask", [128, 128], F32, kind="ExternalInput").ap()
    ident = nc.dram_tensor("ident", [128, 128], F32R, kind="ExternalInput").ap()
    onesc = nc.dram_tensor("onesc", [128, 1], F32R, kind="ExternalInput").ap()
    y = nc.dram_tensor("y", [batch, seq, E], dt_out, kind="ExternalOutput").ap()

    with tile.TileContext(nc) as tc, ExitStack() as ctx:
        consts = ctx.enter_context(tc.tile_pool(name="consts", bufs=1))
        xt_pool = ctx.enter_context(tc.tile_pool(name="xt", bufs=NE + 2))
        big = ctx.enter_context(tc.tile_pool(name="big", bufs=2))
        vsb_pool = ctx.enter_context(tc.tile_pool(name="vsb", bufs=2))
        ex_pool = ctx.enter_context(tc.tile_pool(name="ex", bufs=4))
        ysb_pool = ctx.enter_context(tc.tile_pool(name="ysb", bufs=3))
        small = ctx.enter_context(tc.tile_pool(name="small", bufs=2))
        # PSUM (8 banks): scores 2x2 + ctx 2x1 + shared proj/transpose/y 2x1
        sc_pool = ctx.enter_context(tc.tile_pool(name="scps", bufs=2, space="PSUM"))
        ctx_pool = ctx.enter_context(tc.tile_pool(name="ctxps", bufs=2, space="PSUM"))
        mm_pool = ctx.enter_context(tc.tile_pool(name="mmps", bufs=2, space="PSUM"))

        # ---- constants / weights (resident) ----
        wq_sb = consts.tile([128, E], dt_ind)
        wk_sb = consts.tile([128, E], dt_ind)
        wv_sb = consts.tile([128, E], dt_ind)
        wo_sb = consts.tile([EL, E], F32R)
        tri_sb = consts.tile([128, 128], F32)
        id_sb = consts.tile([128, 128], F32R)
        ones_sb = consts.tile([128, 1], F32R)
        for dst, src in ((wq_sb, wq), (wk_sb, wk), (wv_sb, wv), (wo_sb, wo),
                         (tri_sb, trimask), (id_sb, ident), (ones_sb, onesc)):
            nc.sync.dma_start(dst[:], src)

        rep_cm = tc.For_i(0, n_reps, 1) if n_reps > 1 else None
        if rep_cm is not None:
            rep_cm.__enter__()
        n_y = 0
        for b in range(batch):
            # ---- load xT for this batch ----
            xts = []
            for e in range(NE):
                xt = xt_pool.tile([128, seq], dt_ind, tag="xt")
                nc.sync.dma_start(xt[:], xT[e * 128:(e + 1) * 128, b * seq:(b + 1) * seq])
                xts.append(xt)

            # ---- per-chunk: project q/k/v chunk qc, then attend qc ----
            qT = big.tile([EL, seq], F32R, tag="qT")
            kT = big.tile([EL, seq], F32R, tag="kT")
            vT = big.tile([EL, seq], F32R, tag="vT", bufs=1)
            ctxnT = big.tile([EL, seq], F32R, tag="ctxnT", bufs=1)
            vsb_all = vsb_pool.tile([128, nkt_b, 130], F32R, tag="vsb", name="vsb_all")
            nc.vector.tensor_copy(
                vsb_all.rearrange("p t (h c) -> p (t h) c", c=65)[:, :, 64:65],
                ones_sb.broadcast_to([128, 2 * nkt_b, 1]))
            for qc in range(ncb):
                # q and k projections for this chunk share one 2-bank sc tile
                psqk = sc_pool.tile([128, 2, CH], F32, tag="sc", name="psqk")
                for e in range(NE):
                    nc.tensor.matmul(psqk[:, 0, :], wq_sb[:, ts(e, 128)],
                                     xts[e][:, ts(qc, CH)],
                                     start=(e == 0), stop=(e == NE - 1))
                    nc.tensor.matmul(psqk[:, 1, :], wk_sb[:, ts(e, 128)],
                                     xts[e][:, ts(qc, CH)],
                                     start=(e == 0), stop=(e == NE - 1))
                nc.vector.tensor_copy(qT[:, ts(qc, CH)], psqk[:, 0, :])
                nc.vector.tensor_copy(kT[:, ts(qc, CH)], psqk[:, 1, :])
                # v projection for this chunk
                psv = mm_pool.tile([128, CH], F32, tag="mm", name="psv")
                for e in range(NE):
                    nc.tensor.matmul(psv[:], wv_sb[:, ts(e, 128)],
                                     xts[e][:, ts(qc, CH)],
                                     start=(e == 0), stop=(e == NE - 1))
                nc.vector.tensor_copy(vT[:, ts(qc, CH)], psv[:])
                # transpose this chunk's 4 v t-tiles into [k, (vA|1|vB|1)]
                for tt in range(qc * (CH // 128), (qc + 1) * (CH // 128)):
                    tp = mm_pool.tile([128, 128], F32R, tag="mm", name="tp_ps")
                    nc.tensor.transpose(tp[:], vT[:, ts(tt, 128)], id_sb[:])
                    nc.vector.tensor_copy(
                        vsb_all[:, tt, 0:130].rearrange("p (h c) -> p h c", h=2)[:, :, 0:64],
                        tp.rearrange("p (h c) -> p h c", h=2),
                    )

                nkt = (qc * CH + CH) // KT  # causal: k-tiles 0 .. nkt-1
                cps = [ctx_pool.tile([65, CH], F32, tag="ctx", name=f"ctx_ps{hi}")
                       for hi in range(2)]
                for kt in range(nkt):
                    c0 = max(0, kt * KT - qc * CH)
                    n = CH - c0
                    diag = kt * KT >= qc * CH
                    # both heads' scores into one 2-bank psum tile
                    sc = sc_pool.tile([128, 2, CH], F32, tag="sc", name="sc_ps")
                    for hi in range(2):
                        r0 = hi * 64
                        nc.tensor.matmul(
                            sc[:, hi, 0:n],
                            kT[r0:r0 + 64, ts(kt, KT)],
                            qT[r0:r0 + 64, qc * CH + c0:(qc + 1) * CH],
                            start=True, stop=True,
                        )
                    ex = ex_pool.tile([128, 2, CH], F32R, tag="ex", name="ex")
                    nc.scalar.activation(
                        ex[:, :, 0:n], sc[:, :, 0:n],
                        mybir.ActivationFunctionType.Exp, scale=1.0 / np.sqrt(D))
                    if diag:
                        nc.gpsimd.tensor_mul(
                            ex[:, :, 0:128], ex[:, :, 0:128],
                            tri_sb.unsqueeze(1).broadcast_to([128, 2, 128]))
                    for hi in range(2):
                        nc.tensor.matmul(
                            cps[hi][:, c0:CH],
                            vsb_all[:, kt, hi * 65:(hi + 1) * 65],
                            ex[:, hi, 0:n],
                            start=(kt == 0), stop=(kt == nkt - 1),
                            skip_group_check=True,
                        )
                # normalize: ctxT[0:64] / l (row 64)
                for hi in range(2):
                    r0 = hi * 64
                    rc = small.tile([1, CH], F32, tag="rc", name="rc")
                    nc.vector.reciprocal(rc[:], cps[hi][64:65, :])
                    bc = small.tile([64, CH], F32, tag="bc", name="bc")
                    nc.gpsimd.partition_broadcast(bc[:], rc[:])
                    nc.vector.tensor_mul(
                        ctxnT[r0:r0 + 64, ts(qc, CH)], cps[hi][0:64, :], bc[:])

                # out-projection for this chunk's 4 t-tiles
                for tt in range(qc * (CH // 128), (qc + 1) * (CH // 128)):
                    for eo in range(E // CH):
                        yp = mm_pool.tile([128, CH], F32, tag="mm", name="y_ps")
                        nc.tensor.matmul(
                            yp[:], ctxnT[:, ts(tt, 128)], wo_sb[:, ts(eo, CH)],
                            start=True, stop=True)
                        ysb = ysb_pool.tile([128, CH], dt_out, tag="ysb", name="ysb")
                        if n_y % 2 == 0:
                            nc.vector.tensor_copy(ysb[:], yp[:])
                        else:
                            nc.scalar.copy(ysb[:], yp[:])
                        n_y += 1
                        nc.sync.dma_start(
                            y[b, tt * 128:(tt + 1) * 128, eo * CH:(eo + 1) * CH],
                            ysb[:])

        if rep_cm is not None:
            rep_cm.__exit__(None, None, None)

    nc.compile()
    return nc


def _prep_inputs(x, Wq, Wk, Wv, Wo, dt_in=np.float32):
    """Host-side sharding: transpose x, slice weights per core."""
    batch, seq, _ = x.shape
    xT = np.ascontiguousarray(x.reshape(batch * seq, E).T).astype(dt_in)
    tri = np.triu(np.ones((128, 128), np.float32))  # tri[p, c] = 1 iff p <= c
    identity = np.eye(128, dtype=np.float32)

    def warr(w):  # [E, 128] col-slice -> SBUF layout [128, 8*128]
        return np.ascontiguousarray(
            w.reshape(NE, 128, 128).transpose(1, 0, 2).reshape(128, E)
        ).astype(dt_in)

    in_maps = []
    for i in range(N_CORES):
        cols = slice(i * EL, (i + 1) * EL)
        in_maps.append({
            "xT": xT,
            "wq": warr(Wq[:, cols]),
            "wk": warr(Wk[:, cols]),
            "wv": warr(Wv[:, cols]),
            "wo": np.ascontiguousarray(Wo[cols, :]).astype(np.float32),
            "trimask": tri,
            "ident": identity,
            "onesc": np.ones((128, 1), np.float32),
        })
    return in_maps


_CACHE = {}


def _get_nc(batch, seq, dt_in, dt_out):
    key = (batch, seq, dt_in, dt_out)
    if key not in _CACHE:
        _CACHE[key] = build_attention(batch, seq, dt_in, dt_out)
    return _CACHE[key]


DT_IN = F16   # fp16 x/W transfers; projections accumulate fp32 in PSUM
DT_OUT = F16  # fp16 partial-y transfers; host sums in fp32


def kernel(x, Wq, Wk, Wv, Wo, bo, _trace=False):
    x = np.asarray(x, np.float32)
    batch, seq, _ = x.shape
    nc = _get_nc(batch, seq, DT_IN, DT_OUT)
    in_maps = _prep_inputs(x, np.asarray(Wq), np.asarray(Wk), np.asarray(Wv),
                           np.asarray(Wo),
                           dt_in=np.float16 if DT_IN == F16 else np.float32)
    res = run_bass_kernel_spmd(nc, in_maps, core_ids=list(range(N_CORES)),
                               trace=_trace)
    parts = [res.results[i]["y"].astype(np.float32) for i in range(N_CORES)]
    y = np.sum(parts, axis=0, dtype=np.float32) + np.asarray(bo, np.float32)
    if _trace:
        kernel.last_results = res
    return y


# revision 15
# speedup vs baseline: 25674.9546x; 2.0021x over previous
"""Causal self-attention kernel for Trainium2, sharded over 8 NeuronCores.

Problem (hardcoded): x [2, 2048, 1024] fp32, Wq/Wk/Wv/Wo [1024, 1024], bo [1024].
H = 16 heads, head dim 64.

Sharding: tensor-parallel over heads. Each core owns 2 heads (a 128-wide
column slice of Wq/Wk/Wv and the matching 128-row slice of Wo), computes its
partial out-projection y_i = ctx_i @ Wo[rows_i], and the host sums the 8
partials (the "all-reduce") and adds bo.

Everything on-device is kept in a transposed ("feature-major") layout so that
no on-device transposes of activations or attention probabilities are needed:

  qT, kT        [128 (2 heads x 64 d), T]   from  W.T-slices as lhsT, xT as rhs
  scoresT       [k-tile 128, 2 heads, q-chunk 512] in one 2-bank PSUM tile;
                the two heads' K=64 matmuls sit at base partitions 0/64 so the
                PE row-groups run them concurrently
  expT          exp(scoresT / 8) for both heads in ONE ScalarE activation;
                causal-masked on GPSIMD via a triangular 0/1 mask
  ctxT (+l)     [65, 512]: lhsT = [v | ones] so row 64 accumulates the softmax
                denominator for free; ctxT /= l via partition-broadcast recip
  y partial     lhsT = normalized ctxT t-tile, rhs = Wo local rows

Matmuls use float32r (full-rate fp32 at free dim >= 256).
"""

import sys

import numpy as np

try:
    import concourse.bass as bass  # noqa: F401
except ImportError:  # harness environments without concourse on sys.path
    sys.path.insert(0, "/opt/trn_rl_repo")
    import concourse.bass as bass  # noqa: F401

from contextlib import ExitStack

import concourse.mybir as mybir
import concourse.tile as tile
from concourse import bacc
from concourse.bass import ts
from concourse.bass_utils import run_bass_kernel_spmd

F32 = mybir.dt.float32
F32R = mybir.dt.float32r
F16 = mybir.dt.float16

N_CORES = 8
B, S, E = 2, 2048, 1024
H, D = 16, 64
EL = 128          # local e' width per core (2 heads x 64)
CH = 512          # q-chunk width (one PSUM bank of fp32)
KT = 128          # k-tile width
NE = E // 128     # e-tiles in the contraction dim


def build_attention(batch=B, seq=S, dt_in=F32, dt_out=F32, n_reps=1):
    """Build the per-core Bass program (same program on all 8 cores)."""
    T = batch * seq
    ncb = seq // CH            # q-chunks per batch
    ntt = seq // 128           # t-tiles per batch
    nkt_b = seq // KT          # k-tiles per batch

    nc = bacc.Bacc("TRN2", debug=False, num_devices=N_CORES)

    dt_ind = F32R if dt_in == F32 else dt_in
    dt_i = dt_ind  # internal tile dtype (fp16 fast path, f32r fallback)
    xT = nc.dram_tensor("xT", [E, T], dt_ind, kind="ExternalInput").ap()
    wq = nc.dram_tensor("wq", [128, E], dt_ind, kind="ExternalInput").ap()
    wk = nc.dram_tensor("wk", [128, E], dt_ind, kind="ExternalInput").ap()
    wv = nc.dram_tensor("wv", [128, E], dt_ind, kind="ExternalInput").ap()
    wo = nc.dram_tensor("wo", [EL, E], dt_ind, kind="ExternalInput").ap()
    trimask = nc.dram_tensor("trimask", [128, 128], dt_ind, kind="ExternalInput").ap()
    ident = nc.dram_tensor("ident", [128, 128], dt_ind, kind="ExternalInput").ap()
    onesc = nc.dram_tensor("onesc", [128, 1], dt_ind, kind="ExternalInput").ap()
    onesr = nc.dram_tensor("onesr", [1, 64], F32R, kind="ExternalInput").ap()
    y = nc.dram_tensor("y", [batch, seq, E], dt_out, kind="ExternalOutput").ap()

    with tile.TileContext(nc) as tc, ExitStack() as ctx, \
            nc.allow_low_precision(reason="fp16 internals validated vs fp64 reference"):
        consts = ctx.enter_context(tc.tile_pool(name="consts", bufs=1))
        xt_pool = ctx.enter_context(tc.tile_pool(name="xt", bufs=NE + 2))
        big = ctx.enter_context(tc.tile_pool(name="big", bufs=2))
        vsb_pool = ctx.enter_context(tc.tile_pool(name="vsb", bufs=2))
        ex_pool = ctx.enter_context(tc.tile_pool(name="ex", bufs=4))
        ysb_pool = ctx.enter_context(tc.tile_pool(name="ysb", bufs=3))
        small = ctx.enter_context(tc.tile_pool(name="small", bufs=2))
        # PSUM (8 banks): scores 2x2 + ctx 2x1 + shared proj/transpose/y 2x1
        sc_pool = ctx.enter_context(tc.tile_pool(name="scps", bufs=2, space="PSUM"))
        ctx_pool = ctx.enter_context(tc.tile_pool(name="ctxps", bufs=2, space="PSUM"))
        mm_pool = ctx.enter_context(tc.tile_pool(name="mmps", bufs=2, space="PSUM"))

        # ---- constants / weights (resident) ----
        wq_sb = consts.tile([128, E], dt_ind)
        wk_sb = consts.tile([128, E], dt_ind)
        wv_sb = consts.tile([128, E], dt_ind)
        wo_sb = consts.tile([EL, E], dt_ind)
        tri_sb = consts.tile([128, 128], dt_ind)
        id_sb = consts.tile([128, 128], dt_ind)
        ones_sb = consts.tile([128, 1], dt_ind)
        onesr_sb = consts.tile([1, 64], F32R)
        for dst, src in ((wq_sb, wq), (wk_sb, wk), (wv_sb, wv), (wo_sb, wo),
                         (tri_sb, trimask), (id_sb, ident), (ones_sb, onesc),
                         (onesr_sb, onesr)):
            nc.sync.dma_start(dst[:], src)

        rep_cm = tc.For_i(0, n_reps, 1) if n_reps > 1 else None
        if rep_cm is not None:
            rep_cm.__enter__()
        n_y = 0
        for b in range(batch):
            # ---- load xT for this batch ----
            xts = []
            for e in range(NE):
                xt = xt_pool.tile([128, seq], dt_ind, tag="xt")
                nc.sync.dma_start(xt[:], xT[e * 128:(e + 1) * 128, b * seq:(b + 1) * seq])
                xts.append(xt)

            # ---- per-chunk: project q/k/v chunk qc, then attend qc ----
            qT = big.tile([EL, seq], dt_i, tag="qT")
            kT = big.tile([EL, seq], dt_i, tag="kT")
            vT = big.tile([EL, seq], dt_i, tag="vT", bufs=1)
            ctxnT = big.tile([EL, seq], dt_i, tag="ctxnT", bufs=1)
            vsb_all = vsb_pool.tile([128, nkt_b, 130], dt_i, tag="vsb", name="vsb_all")
            nc.vector.tensor_copy(
                vsb_all.rearrange("p t (h c) -> p (t h) c", c=65)[:, :, 64:65],
                ones_sb.broadcast_to([128, 2 * nkt_b, 1]))
            for qc in range(ncb):
                # q and k projections for this chunk share one 2-bank sc tile
                psqk = sc_pool.tile([128, 2, CH], F32, tag="sc", name="psqk")
                for e in range(NE):
                    nc.tensor.matmul(psqk[:, 0, :], wq_sb[:, ts(e, 128)],
                                     xts[e][:, ts(qc, CH)],
                                     start=(e == 0), stop=(e == NE - 1))
                    nc.tensor.matmul(psqk[:, 1, :], wk_sb[:, ts(e, 128)],
                                     xts[e][:, ts(qc, CH)],
                                     start=(e == 0), stop=(e == NE - 1))
                nc.vector.tensor_copy(qT[:, ts(qc, CH)], psqk[:, 0, :])
                nc.vector.tensor_copy(kT[:, ts(qc, CH)], psqk[:, 1, :])
                # v projection for this chunk
                psv = mm_pool.tile([128, CH], F32, tag="mm", name="psv")
                for e in range(NE):
                    nc.tensor.matmul(psv[:], wv_sb[:, ts(e, 128)],
                                     xts[e][:, ts(qc, CH)],
                                     start=(e == 0), stop=(e == NE - 1))
                nc.vector.tensor_copy(vT[:, ts(qc, CH)], psv[:])
                # transpose this chunk's 4 v t-tiles into [k, (vA|1|vB|1)]
                for tt in range(qc * (CH // 128), (qc + 1) * (CH // 128)):
                    tp = mm_pool.tile([128, 128], dt_i, tag="mm", name="tp_ps")
                    nc.tensor.transpose(tp[:], vT[:, ts(tt, 128)], id_sb[:])
                    nc.vector.tensor_copy(
                        vsb_all[:, tt, 0:130].rearrange("p (h c) -> p h c", h=2)[:, :, 0:64],
                        tp.rearrange("p (h c) -> p h c", h=2),
                    )

                nkt = (qc * CH + CH) // KT  # causal: k-tiles 0 .. nkt-1
                cps = [ctx_pool.tile([65, CH], F32, tag="ctx", name=f"ctx_ps{hi}")
                       for hi in range(2)]
                for kt in range(nkt):
                    c0 = max(0, kt * KT - qc * CH)
                    n = CH - c0
                    diag = kt * KT >= qc * CH
                    # both heads' scores into one 2-bank psum tile
                    sc = sc_pool.tile([128, 2, CH], F32, tag="sc", name="sc_ps")
                    for hi in range(2):
                        r0 = hi * 64
                        nc.tensor.matmul(
                            sc[:, hi, 0:n],
                            kT[r0:r0 + 64, ts(kt, KT)],
                            qT[r0:r0 + 64, qc * CH + c0:(qc + 1) * CH],
                            start=True, stop=True,
                        )
                    ex = ex_pool.tile([128, 2, CH], dt_i, tag="ex", name="ex")
                    nc.scalar.activation(
                        ex[:, :, 0:n], sc[:, :, 0:n],
                        mybir.ActivationFunctionType.Exp, scale=1.0 / np.sqrt(D))
                    if diag:
                        nc.vector.tensor_mul(
                            ex[:, :, 0:128], ex[:, :, 0:128],
                            tri_sb.unsqueeze(1).broadcast_to([128, 2, 128]))
                    for hi in range(2):
                        nc.tensor.matmul(
                            cps[hi][:, c0:CH],
                            vsb_all[:, kt, hi * 65:(hi + 1) * 65],
                            ex[:, hi, 0:n],
                            start=(kt == 0), stop=(kt == nkt - 1),
                            skip_group_check=True,
                        )
                # normalize: ctxT[0:64] / l (row 64)
                for hi in range(2):
                    r0 = hi * 64
                    rc = small.tile([1, CH], F32, tag="rc", name="rc")
                    nc.vector.reciprocal(rc[:], cps[hi][64:65, :])
                    bc = small.tile([64, CH], F32, tag="bc", name="bc")
                    nc.gpsimd.partition_broadcast(bc[:], rc[:])
                    nc.vector.tensor_mul(
                        ctxnT[r0:r0 + 64, ts(qc, CH)], cps[hi][0:64, :], bc[:])

                # out-projection for this chunk's 4 t-tiles
                for tt in range(qc * (CH // 128), (qc + 1) * (CH // 128)):
                    for eo in range(E // CH):
                        yp = mm_pool.tile([128, CH], F32, tag="mm", name="y_ps")
                        nc.tensor.matmul(
                            yp[:], ctxnT[:, ts(tt, 128)], wo_sb[:, ts(eo, CH)],
                            start=True, stop=True)
                        ysb = ysb_pool.tile([128, CH], dt_out, tag="ysb", name="ysb")
                        if n_y % 2 == 0:
                            nc.vector.tensor_copy(ysb[:], yp[:])
                        else:
                            nc.scalar.copy(ysb[:], yp[:])
                        n_y += 1
                        nc.sync.dma_start(
                            y[b, tt * 128:(tt + 1) * 128, eo * CH:(eo + 1) * CH],
                            ysb[:])

        if rep_cm is not None:
            rep_cm.__exit__(None, None, None)

    nc.compile()
    return nc


def _prep_inputs(x, Wq, Wk, Wv, Wo, dt_in=np.float32):
    """Host-side sharding: transpose x, slice weights per core."""
    batch, seq, _ = x.shape
    xT = np.ascontiguousarray(x.reshape(batch * seq, E).T).astype(dt_in)
    tri = np.triu(np.ones((128, 128), np.float32))  # tri[p, c] = 1 iff p <= c
    identity = np.eye(128, dtype=np.float32)

    def warr(w):  # [E, 128] col-slice -> SBUF layout [128, 8*128]
        return np.ascontiguousarray(
            w.reshape(NE, 128, 128).transpose(1, 0, 2).reshape(128, E)
        ).astype(dt_in)

    in_maps = []
    for i in range(N_CORES):
        cols = slice(i * EL, (i + 1) * EL)
        in_maps.append({
            "xT": xT,
            "wq": warr(Wq[:, cols]),
            "wk": warr(Wk[:, cols]),
            "wv": warr(Wv[:, cols]),
            "wo": np.ascontiguousarray(Wo[cols, :]).astype(dt_in),
            "trimask": tri.astype(dt_in),
            "ident": identity.astype(dt_in),
            "onesc": np.ones((128, 1), dt_in),
            "onesr": np.ones((1, 64), np.float32),
        })
    return in_maps


_CACHE = {}


def _get_nc(batch, seq, dt_in, dt_out):
    key = (batch, seq, dt_in, dt_out)
    if key not in _CACHE:
        _CACHE[key] = build_attention(batch, seq, dt_in, dt_out)
    return _CACHE[key]


DT_IN = F16   # fp16 x/W transfers; projections accumulate fp32 in PSUM
DT_OUT = F16  # fp16 partial-y transfers; host sums in fp32


def kernel(x, Wq, Wk, Wv, Wo, bo, _trace=False):
    x = np.asarray(x, np.float32)
    batch, seq, _ = x.shape
    nc = _get_nc(batch, seq, DT_IN, DT_OUT)
    in_maps = _prep_inputs(x, np.asarray(Wq), np.asarray(Wk), np.asarray(Wv),
                           np.asarray(Wo),
                           dt_in=np.float16 if DT_IN == F16 else np.float32)
    res = run_bass_kernel_spmd(nc, in_maps, core_ids=list(range(N_CORES)),
                               trace=_trace)
    parts = [res.results[i]["y"].astype(np.float32) for i in range(N_CORES)]
    y = np.sum(parts, axis=0, dtype=np.float32) + np.asarray(bo, np.float32)
    if _trace:
        kernel.last_results = res
    return y


# revision 16
# speedup vs baseline: 27260.8269x; 1.0618x over previous
"""Causal self-attention kernel for Trainium2, sharded over 8 NeuronCores.

Problem (hardcoded): x [2, 2048, 1024] fp32, Wq/Wk/Wv/Wo [1024, 1024], bo [1024].
H = 16 heads, head dim 64.

Sharding: tensor-parallel over heads. Each core owns 2 heads (a 128-wide
column slice of Wq/Wk/Wv and the matching 128-row slice of Wo), computes its
partial out-projection y_i = ctx_i @ Wo[rows_i], and the host sums the 8
partials (the "all-reduce") and adds bo.

Everything on-device is kept in a transposed ("feature-major") layout so that
no on-device transposes of activations or attention probabilities are needed:

  qT, kT        [128 (2 heads x 64 d), T]   from  W.T-slices as lhsT, xT as rhs
  scoresT       [k-tile 128, 2 heads, q-chunk 512] in one 2-bank PSUM tile;
                the two heads' K=64 matmuls sit at base partitions 0/64 so the
                PE row-groups run them concurrently
  expT          exp(scoresT / 8) for both heads in ONE ScalarE activation;
                causal-masked on GPSIMD via a triangular 0/1 mask
  ctxT (+l)     [65, 512]: lhsT = [v | ones] so row 64 accumulates the softmax
                denominator for free; ctxT /= l via partition-broadcast recip
  y partial     lhsT = normalized ctxT t-tile, rhs = Wo local rows

Matmuls use float32r (full-rate fp32 at free dim >= 256).
"""

import sys

import numpy as np

try:
    import concourse.bass as bass  # noqa: F401
except ImportError:  # harness environments without concourse on sys.path
    sys.path.insert(0, "/opt/trn_rl_repo")
    import concourse.bass as bass  # noqa: F401

from contextlib import ExitStack

import concourse.mybir as mybir
import concourse.tile as tile
from concourse import bacc
from concourse.bass import ts
from concourse.bass_utils import run_bass_kernel_spmd

F32 = mybir.dt.float32
F32R = mybir.dt.float32r
F16 = mybir.dt.float16

N_CORES = 8
B, S, E = 2, 2048, 1024
H, D = 16, 64
EL = 128          # local e' width per core (2 heads x 64)
CH = 512          # q-chunk width (one PSUM bank of fp32)
KT = 128          # k-tile width
NE = E // 128     # e-tiles in the contraction dim


def build_attention(batch=B, seq=S, dt_in=F32, dt_out=F32, n_reps=1):
    """Build the per-core Bass program (same program on all 8 cores)."""
    T = batch * seq
    ncb = seq // CH            # q-chunks per batch
    ntt = seq // 128           # t-tiles per batch
    nkt_b = seq // KT          # k-tiles per batch

    nc = bacc.Bacc("TRN2", debug=False, num_devices=N_CORES)

    dt_ind = F32R if dt_in == F32 else dt_in
    dt_i = dt_ind  # internal tile dtype (fp16 fast path, f32r fallback)
    xT = nc.dram_tensor("xT", [E, T], dt_ind, kind="ExternalInput").ap()
    wq = nc.dram_tensor("wq", [128, E], dt_ind, kind="ExternalInput").ap()
    wk = nc.dram_tensor("wk", [128, E], dt_ind, kind="ExternalInput").ap()
    wv = nc.dram_tensor("wv", [128, E], dt_ind, kind="ExternalInput").ap()
    wo = nc.dram_tensor("wo", [EL, E], dt_ind, kind="ExternalInput").ap()
    trimask = nc.dram_tensor("trimask", [128, 128], dt_ind, kind="ExternalInput").ap()
    ident = nc.dram_tensor("ident", [128, 128], dt_ind, kind="ExternalInput").ap()
    onesc = nc.dram_tensor("onesc", [128, 1], dt_ind, kind="ExternalInput").ap()
    onesr = nc.dram_tensor("onesr", [1, 64], F32R, kind="ExternalInput").ap()
    y = nc.dram_tensor("y", [batch, seq, E], dt_out, kind="ExternalOutput").ap()

    with tile.TileContext(nc) as tc, ExitStack() as ctx, \
            nc.allow_low_precision(reason="fp16 internals validated vs fp64 reference"):
        consts = ctx.enter_context(tc.tile_pool(name="consts", bufs=1))
        xt_pool = ctx.enter_context(tc.tile_pool(name="xt", bufs=NE + 4))
        big = ctx.enter_context(tc.tile_pool(name="big", bufs=2))
        vsb_pool = ctx.enter_context(tc.tile_pool(name="vsb", bufs=2))
        ex_pool = ctx.enter_context(tc.tile_pool(name="ex", bufs=6))
        ysb_pool = ctx.enter_context(tc.tile_pool(name="ysb", bufs=4))
        small = ctx.enter_context(tc.tile_pool(name="small", bufs=3))
        # PSUM (8 banks): scores 2x2 + ctx 2x1 + shared proj/transpose/y 2x1
        sc_pool = ctx.enter_context(tc.tile_pool(name="scps", bufs=2, space="PSUM"))
        ctx_pool = ctx.enter_context(tc.tile_pool(name="ctxps", bufs=2, space="PSUM"))
        mm_pool = ctx.enter_context(tc.tile_pool(name="mmps", bufs=2, space="PSUM"))

        # ---- constants / weights (resident) ----
        wq_sb = consts.tile([128, E], dt_ind)
        wk_sb = consts.tile([128, E], dt_ind)
        wv_sb = consts.tile([128, E], dt_ind)
        wo_sb = consts.tile([EL, E], dt_ind)
        tri_sb = consts.tile([128, 128], dt_ind)
        id_sb = consts.tile([128, 128], dt_ind)
        ones_sb = consts.tile([128, 1], dt_ind)
        onesr_sb = consts.tile([1, 64], F32R)
        for dst, src in ((wq_sb, wq), (wk_sb, wk), (wv_sb, wv), (wo_sb, wo),
                         (tri_sb, trimask), (id_sb, ident), (ones_sb, onesc),
                         (onesr_sb, onesr)):
            nc.sync.dma_start(dst[:], src)

        rep_cm = tc.For_i(0, n_reps, 1) if n_reps > 1 else None
        if rep_cm is not None:
            rep_cm.__enter__()
        n_y = 0
        for b in range(batch):
            # ---- load xT for this batch ----
            xts = []
            for e in range(NE):
                xt = xt_pool.tile([128, seq], dt_ind, tag="xt")
                nc.sync.dma_start(xt[:], xT[e * 128:(e + 1) * 128, b * seq:(b + 1) * seq])
                xts.append(xt)

            # ---- per-chunk: project q/k/v chunk qc, then attend qc ----
            qT = big.tile([EL, seq], dt_i, tag="qT")
            kT = big.tile([EL, seq], dt_i, tag="kT")
            vT = big.tile([EL, seq], dt_i, tag="vT", bufs=1)
            ctxnT = big.tile([EL, seq], dt_i, tag="ctxnT", bufs=1)
            vsb_all = vsb_pool.tile([128, nkt_b, 130], dt_i, tag="vsb", name="vsb_all")
            nc.vector.tensor_copy(
                vsb_all.rearrange("p t (h c) -> p (t h) c", c=65)[:, :, 64:65],
                ones_sb.broadcast_to([128, 2 * nkt_b, 1]))
            for qc in range(ncb):
                # q and k projections for this chunk share one 2-bank sc tile
                psqk = sc_pool.tile([128, 2, CH], F32, tag="sc", name="psqk")
                for e in range(NE):
                    nc.tensor.matmul(psqk[:, 0, :], wq_sb[:, ts(e, 128)],
                                     xts[e][:, ts(qc, CH)],
                                     start=(e == 0), stop=(e == NE - 1))
                    nc.tensor.matmul(psqk[:, 1, :], wk_sb[:, ts(e, 128)],
                                     xts[e][:, ts(qc, CH)],
                                     start=(e == 0), stop=(e == NE - 1))
                nc.vector.tensor_copy(qT[:, ts(qc, CH)], psqk[:, 0, :])
                nc.vector.tensor_copy(kT[:, ts(qc, CH)], psqk[:, 1, :])
                # v projection for this chunk
                psv = mm_pool.tile([128, CH], F32, tag="mm", name="psv")
                for e in range(NE):
                    nc.tensor.matmul(psv[:], wv_sb[:, ts(e, 128)],
                                     xts[e][:, ts(qc, CH)],
                                     start=(e == 0), stop=(e == NE - 1))
                nc.vector.tensor_copy(vT[:, ts(qc, CH)], psv[:])
                # transpose this chunk's 4 v t-tiles into [k, (vA|1|vB|1)]
                for tt in range(qc * (CH // 128), (qc + 1) * (CH // 128)):
                    tp = mm_pool.tile([128, 128], dt_i, tag="mm", name="tp_ps")
                    nc.tensor.transpose(tp[:], vT[:, ts(tt, 128)], id_sb[:])
                    nc.vector.tensor_copy(
                        vsb_all[:, tt, 0:130].rearrange("p (h c) -> p h c", h=2)[:, :, 0:64],
                        tp.rearrange("p (h c) -> p h c", h=2),
                    )

                nkt = (qc * CH + CH) // KT  # causal: k-tiles 0 .. nkt-1
                cps = [ctx_pool.tile([65, CH], F32, tag="ctx", name=f"ctx_ps{hi}")
                       for hi in range(2)]
                for kt in range(nkt):
                    c0 = max(0, kt * KT - qc * CH)
                    n = CH - c0
                    diag = kt * KT >= qc * CH
                    # both heads' scores into one 2-bank psum tile
                    sc = sc_pool.tile([128, 2, CH], F32, tag="sc", name="sc_ps")
                    for hi in range(2):
                        r0 = hi * 64
                        nc.tensor.matmul(
                            sc[:, hi, 0:n],
                            kT[r0:r0 + 64, ts(kt, KT)],
                            qT[r0:r0 + 64, qc * CH + c0:(qc + 1) * CH],
                            start=True, stop=True,
                        )
                    ex = ex_pool.tile([128, 2, CH], dt_i, tag="ex", name="ex")
                    nc.scalar.activation(
                        ex[:, :, 0:n], sc[:, :, 0:n],
                        mybir.ActivationFunctionType.Exp, scale=1.0 / np.sqrt(D))
                    if diag:
                        nc.vector.tensor_mul(
                            ex[:, :, 0:128], ex[:, :, 0:128],
                            tri_sb.unsqueeze(1).broadcast_to([128, 2, 128]))
                    for hi in range(2):
                        nc.tensor.matmul(
                            cps[hi][:, c0:CH],
                            vsb_all[:, kt, hi * 65:(hi + 1) * 65],
                            ex[:, hi, 0:n],
                            start=(kt == 0), stop=(kt == nkt - 1),
                            skip_group_check=True,
                        )
                # normalize: ctxT[0:64] / l (row 64)
                for hi in range(2):
                    r0 = hi * 64
                    rc = small.tile([1, CH], F32, tag="rc", name="rc")
                    nc.vector.reciprocal(rc[:], cps[hi][64:65, :])
                    bc = small.tile([64, CH], F32, tag="bc", name="bc")
                    nc.gpsimd.partition_broadcast(bc[:], rc[:])
                    nc.vector.tensor_mul(
                        ctxnT[r0:r0 + 64, ts(qc, CH)], cps[hi][0:64, :], bc[:])

                # out-projection for this chunk's 4 t-tiles
                for tt in range(qc * (CH // 128), (qc + 1) * (CH // 128)):
                    for eo in range(E // CH):
                        yp = mm_pool.tile([128, CH], F32, tag="mm", name="y_ps")
                        nc.tensor.matmul(
                            yp[:], ctxnT[:, ts(tt, 128)], wo_sb[:, ts(eo, CH)],
                            start=True, stop=True)
                        ysb = ysb_pool.tile([128, CH], dt_out, tag="ysb", name="ysb")
                        if n_y % 2 == 0:
                            nc.vector.tensor_copy(ysb[:], yp[:])
                        else:
                            nc.scalar.copy(ysb[:], yp[:])
                        n_y += 1
                        nc.sync.dma_start(
                            y[b, tt * 128:(tt + 1) * 128, eo * CH:(eo + 1) * CH],
                            ysb[:])

        if rep_cm is not None:
            rep_cm.__exit__(None, None, None)

    nc.compile()
    return nc


def _prep_inputs(x, Wq, Wk, Wv, Wo, dt_in=np.float32):
    """Host-side sharding: transpose x, slice weights per core."""
    batch, seq, _ = x.shape
    xT = np.ascontiguousarray(x.reshape(batch * seq, E).T).astype(dt_in)
    tri = np.triu(np.ones((128, 128), np.float32))  # tri[p, c] = 1 iff p <= c
    identity = np.eye(128, dtype=np.float32)

    def warr(w):  # [E, 128] col-slice -> SBUF layout [128, 8*128]
        return np.ascontiguousarray(
            w.reshape(NE, 128, 128).transpose(1, 0, 2).reshape(128, E)
        ).astype(dt_in)

    in_maps = []
    for i in range(N_CORES):
        cols = slice(i * EL, (i + 1) * EL)
        in_maps.append({
            "xT": xT,
            "wq": warr(Wq[:, cols]),
            "wk": warr(Wk[:, cols]),
            "wv": warr(Wv[:, cols]),
            "wo": np.ascontiguousarray(Wo[cols, :]).astype(dt_in),
            "trimask": tri.astype(dt_in),
            "ident": identity.astype(dt_in),
            "onesc": np.ones((128, 1), dt_in),
            "onesr": np.ones((1, 64), np.float32),
        })
    return in_maps


_CACHE = {}


def _get_nc(batch, seq, dt_in, dt_out):
    key = (batch, seq, dt_in, dt_out)
    if key not in _CACHE:
        _CACHE[key] = build_attention(batch, seq, dt_in, dt_out)
    return _CACHE[key]


DT_IN = F16   # fp16 x/W transfers; projections accumulate fp32 in PSUM
DT_OUT = F16  # fp16 partial-y transfers; host sums in fp32


def kernel(x, Wq, Wk, Wv, Wo, bo, _trace=False):
    x = np.asarray(x, np.float32)
    batch, seq, _ = x.shape
    nc = _get_nc(batch, seq, DT_IN, DT_OUT)
    in_maps = _prep_inputs(x, np.asarray(Wq), np.asarray(Wk), np.asarray(Wv),
                           np.asarray(Wo),
                           dt_in=np.float16 if DT_IN == F16 else np.float32)
    res = run_bass_kernel_spmd(nc, in_maps, core_ids=list(range(N_CORES)),
                               trace=_trace)
    parts = [res.results[i]["y"].astype(np.float32) for i in range(N_CORES)]
    y = np.sum(parts, axis=0, dtype=np.float32) + np.asarray(bo, np.float32)
    if _trace:
        kernel.last_results = res
    return y
